# revision 1
# baseline (speedup 1.0000x reference)
"""Self-contained Trainium kernel for nn_B_29729763623191 (axial/sparse attention block).

Contract: kernel(**inputs) takes FULL unsharded inputs, returns FULL output.
Sharding: data-parallel over batch (axis 0 of x) across NeuronCores; all
weights replicated. The axial attentions are independent per batch item, so
no cross-device communication is needed in forward (matches sharding_hint).

Hardcoded problem shapes: x [4, 64, 192, 192] f32, C=64, HEADS=2.
"""

import numpy as np
import jax
import jax.numpy as jnp

DIM = 64
HEADS = 2
BATCH, H, W = 4, 192, 192

_PARAM_NAMES = (
    'pw_w', 'dw_w', 'dw_b', 'conv2_w', 'conv2_b', 'conv0_w', 'conv0_b',
    'att_q_w', 'att_k_w', 'att_v_w', 'att_proj_w', 'temperature',
    'row_q_w', 'row_k_w', 'row_v_w', 'row_gamma',
    'col_q_w', 'col_k_w', 'col_v_w', 'col_gamma',
    'conv_w', 'conv_b',
)


def _conv1x1(x, w, b=None):
    # x: [c,h,w], w: [o,c]
    y = jnp.einsum('chw,oc->ohw', x, w)
    if b is not None:
        y = y + b[:, None, None]
    return y


def _gelu(x):
    return jax.nn.gelu(x, approximate=False)


def _depthwise3x3(t, dw_w, dw_b):
    # t: [c,h,w]; dw_w: [c,1,3,3]; 'same' padding via 9 shifted mul-adds —
    # lowers to plain elementwise ops on the NeuronCore (no grouped-conv path).
    c, h, w = t.shape
    tp = jnp.pad(t, ((0, 0), (1, 1), (1, 1)))
    acc = jnp.zeros_like(t)
    for di in range(3):
        for dj in range(3):
            acc = acc + dw_w[:, 0, di, dj][:, None, None] * tp[:, di:di + h, dj:dj + w]
    return acc + dw_b[:, None, None]


def _forward_one(x, pw_w, dw_w, dw_b, conv2_w, conv2_b, conv0_w, conv0_b,
                 att_q_w, att_k_w, att_v_w, att_proj_w, temperature,
                 row_q_w, row_k_w, row_v_w, row_gamma,
                 col_q_w, col_k_w, col_v_w, col_gamma,
                 conv_w, conv_b):
    # x: [c,h,w] — one batch item, runs entirely on one NeuronCore.
    c, h, w = x.shape

    # --- BSConvU -> GELU -> conv_2, plus conv_0 skip ---
    t = _conv1x1(x, pw_w)
    t = _depthwise3x3(t, dw_w, dw_b)
    x1 = _conv1x1(_gelu(t), conv2_w, conv2_b) + _conv1x1(x, conv0_w, conv0_b)

    # --- channel self-attention (out_1) ---
    ch = c // HEADS
    n = h * w
    q = _conv1x1(x1, att_q_w).reshape(HEADS, ch, n)
    k = _conv1x1(x1, att_k_w).reshape(HEADS, ch, n)
    v = _conv1x1(x1, att_v_w).reshape(HEADS, ch, n)
    q = q / jnp.maximum(jnp.linalg.norm(q, axis=-1, keepdims=True), 1e-12)
    k = k / jnp.maximum(jnp.linalg.norm(k, axis=-1, keepdims=True), 1e-12)
    attn = jnp.einsum('gcn,gdn->gcd', q, k) * temperature
    attn = jax.nn.softmax(attn, axis=-1)
    out1 = jnp.einsum('gcd,gdn->gcn', attn, v).reshape(c, h, w)
    out1 = _conv1x1(out1, att_proj_w)

    # --- row attention over width, per row (out_2) ---
    Q = _conv1x1(x1, row_q_w)
    K = _conv1x1(x1, row_k_w)
    V = _conv1x1(out1, row_v_w)
    s = jnp.einsum('chi,chj->hij', Q, K)            # [h,w,w]
    a = jax.nn.softmax(s, axis=-1)
    out2 = jnp.einsum('chj,hij->chi', V, a)
    out2 = row_gamma * out2 + x1

    # --- col attention over height, per column (out_3) ---
    Q = _conv1x1(x1, col_q_w)
    K = _conv1x1(x1, col_k_w)
    V = _conv1x1(out1, col_v_w)
    s = jnp.einsum('ciw,cjw->wij', Q, K)            # [w,h,h]
    a = jax.nn.softmax(s, axis=-1)
    out3 = jnp.einsum('cjw,wij->ciw', V, a)
    out3 = col_gamma * out3 + x1

    out = jnp.concatenate([out1, out2, out3], axis=0)
    return _gelu(_conv1x1(out, conv_w, conv_b))


_PMAP_CACHE = {}


def _get_pmapped(n_shard):
    fn = _PMAP_CACHE.get(n_shard)
    if fn is None:
        devs = jax.devices()[:n_shard]
        in_axes = (0,) + (None,) * len(_PARAM_NAMES)
        fn = jax.pmap(_forward_one, devices=devs, in_axes=in_axes)
        _PMAP_CACHE[n_shard] = fn
    return fn


def kernel(**inputs):
    x = np.asarray(inputs['x'], dtype=np.float32)
    params = tuple(np.asarray(inputs[k], dtype=np.float32) for k in _PARAM_NAMES)
    b = x.shape[0]
    n_dev = len(jax.devices())
    n_shard = min(b, n_dev)
    try:
        fn = _get_pmapped(n_shard)
        if b == n_shard:
            out = fn(x, *params)
            res = np.asarray(out, dtype=np.float32)
        else:
            chunks = []
            for s in range(0, b, n_shard):
                xs = x[s:s + n_shard]
                pad = n_shard - xs.shape[0]
                if pad:
                    xs = np.concatenate([xs, np.zeros((pad,) + xs.shape[1:], xs.dtype)])
                o = np.asarray(fn(xs, *params))
                chunks.append(o[:n_shard - pad] if pad else o)
            res = np.concatenate(chunks, axis=0).astype(np.float32)
        return res
    except Exception:
        # Fallback: correctness-preserving host path if device compile/run fails.
        cpu = jax.devices('cpu')[0]
        with jax.default_device(cpu):
            f = jax.jit(jax.vmap(_forward_one, in_axes=(0,) + (None,) * len(_PARAM_NAMES)))
            out = f(jnp.asarray(x), *[jnp.asarray(p) for p in params])
            return np.asarray(out, dtype=np.float32)
